# revision 23
# baseline (speedup 1.0000x reference)
"""MoE grouped linear (DMoELinear) on 8 Trainium2 NeuronCores.

Expert-parallel sharding: tokens are sorted by expert id, so expert e's
tokens form one contiguous slice. Core e receives expert e's tokens
(padded to a uniform capacity C = max group size, so all cores run one
SPMD NEFF), expert e's weight and bias, and computes
    yT_e = (x_e @ W_e.T).T.bf16 + b_e.bf16
with the weight block as the stationary matmul operand and tokens as
the moving free dim.

v7 schedule (trace-driven). The 2-ring DMA system (~190GB/s per ring,
one dma_start = ~0.6us issuing-engine time, 4-queue rotation per
engine) cannot feed the PE in pure bf16 (the in-stream totals 6.4MB),
so the weights for dbs 2..15 ship INT8 with a per-output-row scale
(max-abs/127; measured rel-err ~0.85% vs the 2e-2 gate):
  - int8 -> bf16 up-conversion runs on the ACT engine via
    ACTIVATE-Copy (~115 G elem/s under load; the DVE/Pool
    tensor_scalar path measures ~9 G elem/s — unusable), which is
    why ALL psum evictions move to the DVE;
  - the matmul runs on integer-valued bf16 weights (products and
    f32 PSUM accumulation are exact) and the dequant scale folds
    into the eviction (DVE tensor_scalar mult+add; ACT activation
    scale+bias for the drain pieces) — zero extra instructions;
  - x and the trickle weights (dbs 0/1) stay bf16: the trickle is
    paced by its own DMA either way and ACT cannot also convert x
    tiles at the trickle's consumption rate.

PE schedule: 30 warm matmuls flip the HAM clock gate while the first
tiles land; db0/db1 interleave k-major (trickle, DMA-paced); dbs 2..14
run chunk-major (c0's k-loop first, evicted while c1/c2 interleave) so
PSUM banks retire mid-db; db15 splits its middle 512 region into
256+128+128 pieces with per-piece PSUM tiles so after the last matmul
only two parallel 128-wide evictions and one 64KB DMA remain.
"""

import numpy as np
import ml_dtypes

N_TOK, D_IN, D_OUT, N_EXP = 8192, 1024, 2048, 8
N_CORES = 8
P = 128
NFREE = 512  # max matmul moving free dim (one PSUM bank of f32)

X_CLIP = 4.0  # x quantization clip (in sigmas; x ~ N(0,1))

BF16 = ml_dtypes.bfloat16

_nc_cache: dict[int, object] = {}


def _chunks(C):
    out = []
    off = 0
    while off < C:
        cw = min(NFREE, C - off)
        out.append((off, cw))
        off += cw
    return out


def _build_bass(C: int):
    """Emit the per-core Bass/Tile kernel for token capacity C."""
    import concourse.bass as bass  # noqa: F401  (registers engines)
    import concourse.mybir as mybir
    import concourse.tile as tile
    from concourse import bacc

    dt = mybir.dt
    KT = D_IN // P      # 8 contraction tiles
    DB = D_OUT // P     # 16 output-row blocks
    KW = KT * P         # columns per db block in the flat weight (1024)
    chunks = _chunks(C)
    chunk_of_db = {db: chunks for db in range(DB)}
    Ident = mybir.ActivationFunctionType.Identity
    MUL, ADD = mybir.AluOpType.mult, mybir.AluOpType.add

    nc = bacc.Bacc("TRN2", target_bir_lowering=False)

    # x partition-flat bf16: row p, col ki*C + c = x[token c, ki*128+p]
    xf_d = nc.dram_tensor("xf", [P, KT * C], dt.bfloat16, kind="ExternalInput")
    # dbs 0/1 weights bf16: row p, col db*1024 + kt*128 + d
    w01_d = nc.dram_tensor("w01", [P, 2 * KW], dt.bfloat16, kind="ExternalInput")
    # dbs 2..15 weights int8, same layout with db index shifted by 2
    w8_d = nc.dram_tensor("w8f", [P, (DB - 2) * KW], dt.int8, kind="ExternalInput")
    bias_d = nc.dram_tensor("biasp", [P, DB], dt.float32, kind="ExternalInput")
    sc_d = nc.dram_tensor("scp", [P, DB], dt.float32, kind="ExternalInput")
    y_d = nc.dram_tensor("yT", [D_OUT, C], dt.bfloat16, kind="ExternalOutput")

    with tile.TileContext(nc) as tc:
        with (
            tc.tile_pool(name="persist", bufs=1) as ppool,
            tc.tile_pool(name="yout", bufs=4) as ypool,
            tc.tile_pool(name="psum", bufs=8, space="PSUM") as pspool,
        ):
            x_tiles = [
                ppool.tile([P, C], dt.bfloat16, name=f"x{ki}", tag=f"x{ki}")
                for ki in range(KT)
            ]

            def x_sl(ki, off, cw):
                return x_tiles[ki][:, off:off + cw]

            w_s = [
                ppool.tile([P, KW], dt.bfloat16, name=f"w{db}", tag=f"w{db}")
                for db in range(4)
            ]
            w8_s = {
                db: ppool.tile([P, KW], dt.int8, name=f"w8_{db}", tag=f"w8_{db}")
                for db in (2, 3)
            }
            packs = [
                ppool.tile([P, 2 * KW], dt.bfloat16, name=f"wp{g}", tag=f"wp{g}")
                for g in range(2, 8)
            ]
            packs8 = [
                ppool.tile([P, 2 * KW], dt.int8, name=f"wp8_{g}", tag=f"wp8_{g}")
                for g in range(2, 8)
            ]
            bt = ppool.tile([P, DB], dt.float32, name="bias", tag="bias")
            st = ppool.tile([P, DB], dt.float32, name="scale", tag="scale")

            def lhsT(db, ki):
                if db < 4:
                    return w_s[db][:, ki * P:(ki + 1) * P]
                g = db // 2
                off = (db - 2 * g) * KW + ki * P
                return packs[g - 2][:, off:off + P]

            # ── DMA schedule: two HWDGE rings ────────────────────────
            # A = sync, B = scalar. x tiles 277KB bf16, w0/w1 bf16
            # half-slabs ordered by first use, everything else int8
            # (halving its bytes): the in-stream totals ~4.8MB and the
            # trickle's tail is only mildly DMA-paced.
            A, B = nc.sync, nc.scalar

            def xdma(ki, eng):
                eng.dma_start(x_tiles[ki][:], xf_d[:, ki * C:(ki + 1) * C])

            # x and w0/w1 slabs alternate so every trickle step's
            # operands land just ahead of its deadline; w8_2 rides B
            # mid-stream (its ACT convert must beat db2's start), w8_3
            # and all int8 packs ride A; bias/scale late on B.
            xdma(1, A)
            xdma(0, B)
            A.dma_start(w_s[0][:, 0:4 * P], w01_d[:, 0:4 * P])
            B.dma_start(w_s[1][:, 0:4 * P], w01_d[:, KW:KW + 4 * P])
            xdma(3, A)
            xdma(2, B)
            A.dma_start(w_s[0][:, 4 * P:KW], w01_d[:, 4 * P:KW])
            B.dma_start(w_s[1][:, 4 * P:KW], w01_d[:, KW + 4 * P:2 * KW])
            xdma(5, A)
            xdma(4, B)
            B.dma_start(w8_s[2][:], w8_d[:, 0:KW])
            xdma(7, A)
            xdma(6, B)
            A.dma_start(w8_s[3][:], w8_d[:, KW:2 * KW])
            B.dma_start(bt[:], bias_d[:])
            B.dma_start(st[:], sc_d[:])

            # ── int8 → bf16 weight converts on the ACT engine ────────
            # ACTIVATE-Copy casts exactly (values are integers <=127)
            # at ~115 G elem/s under load: ~1.2us per db, ~2.3us per
            # pack. Emitted BEFORE the pack DMA issues so the scalar
            # engine runs w2's convert the moment the slab lands
            # (~14us) instead of after its whole issue queue.
            nc.scalar.copy(w_s[2][:], w8_s[2][:])
            nc.scalar.copy(w_s[3][:], w8_s[3][:])
            for g in range(2, 8):
                A.dma_start(
                    packs8[g - 2][:],
                    w8_d[:, (2 * g - 2) * KW:(2 * g) * KW],
                )
            for g in range(2, 8):
                nc.scalar.copy(packs[g - 2][:], packs8[g - 2][:])

            # ── PE warmup: flip the HAM clock gate while tiles land ──
            warm = ppool.tile([P, P], dt.bfloat16, name="warm", tag="warm")
            nc.vector.memset(warm[:], 0.0)
            wps = pspool.tile([P, P], dt.float32, name="wps", tag="ps")

            def warm_mm(n):
                for _ in range(n):
                    nc.tensor.matmul(wps[:], warm[:], warm[:], start=True, stop=True)

            warm_mm(30)

            all_psums = {}

            def alloc_chunk(db, j):
                _, cw = chunk_of_db[db][j]
                return pspool.tile([P, cw], dt.float32, name=f"ps{db}_{j}", tag="ps")

            def alloc_psums(db, chunks_j=None):
                js = chunks_j or range(len(chunk_of_db[db]))
                cur = all_psums.setdefault(db, {})
                for j in js:
                    cur[j] = alloc_chunk(db, j)

            def emit_mm(db, ki, j):
                off, cw = chunk_of_db[db][j]
                nc.tensor.matmul(
                    all_psums[db][j][:, :cw],
                    lhsT(db, ki),
                    x_sl(ki, off, cw),
                    start=(ki == 0),
                    stop=(ki == KT - 1),
                )

            def emit_mms(db, ki, chunks_j=None):
                for j in chunks_j or range(len(chunk_of_db[db])):
                    emit_mm(db, ki, j)

            ep = 0
            ysbs = {}

            def new_ysb(db):
                ysbs[db] = ypool.tile([P, C], dt.bfloat16, name="ysb", tag="ysb")
                return ysbs[db]

            def ev_act(dst, src, db):
                nc.scalar.activation(
                    dst, src, Ident, bias=bt[:, db:db + 1], scale=st[:, db:db + 1]
                )

            def ev_dve(dst, src, db):
                nc.vector.tensor_scalar(
                    dst, src, st[:, db:db + 1], bt[:, db:db + 1], op0=MUL, op1=ADD
                )

            def evict_chunk(db, j, ysb):
                # all steady-state evictions ride the DVE: the ACT
                # engine is busy with int8 weight converts until ~33us.
                nonlocal ep
                off, cw = chunk_of_db[db][j]
                ev_dve(ysb[:, off:off + cw], all_psums[db][j][:, :cw], db)
                ep += 1

            def evict(db):
                ysb = new_ysb(db)
                for j in range(len(chunk_of_db[db])):
                    evict_chunk(db, j, ysb)
                return ysb

            def ydma(db, ysb):
                eng = nc.sync if db % 2 == 0 else nc.scalar
                eng.dma_start(y_d[db * P:(db + 1) * P, :], ysb[:])

            # ── Trickle phase ────────────────────────────────────────
            # db0/db1 interleaved by k-step; db1 one step behind so
            # db0's k7 chunks finish (and their PSUM banks evict) while
            # db1's tail runs.
            STAG = 1
            alloc_psums(0)
            alloc_psums(1)
            for step in range(KT + STAG):
                if step < KT:
                    emit_mms(0, step)
                if step >= STAG:
                    emit_mms(1, step - STAG)
            ydma(0, evict(0))
            ydma(1, evict(1))

            # ── dbs 2..14: chunk-major ───────────────────────────────
            # c0 runs its whole k-loop first and evicts while c1/c2
            # (interleaved so the narrow chunk's LDWEIGHTS hide under
            # the 512-wide drains) are still computing. PSUM banks
            # retire mid-db instead of piling up at db boundaries.
            for db in range(2, DB - 1):
                alloc_psums(db)
                ncks = len(chunk_of_db[db])
                for ki in range(KT):
                    emit_mm(db, ki, 0)
                ysb = new_ysb(db)
                evict_chunk(db, 0, ysb)
                for ki in range(KT):
                    for j in range(1, ncks):
                        emit_mm(db, ki, j)
                for j in range(1, ncks):
                    evict_chunk(db, j, ysb)
                if db == DB - 2:
                    # per-chunk DMAs on both rings so the tail pipelines
                    for j, (off, cw) in enumerate(chunk_of_db[db]):
                        eng = nc.sync if (db + j) % 2 == 0 else nc.scalar
                        eng.dma_start(
                            y_d[db * P:(db + 1) * P, off:off + cw],
                            ysb[:, off:off + cw],
                        )
                else:
                    ydma(db, ysb)

            # ── db15: ordered so the drain is minimal ────────────────
            # (c0, tail-chunk) interleaved first — both evicted and
            # DMA'd while the middle 512 region computes as 256+128+128
            # pieces with per-piece PSUM tiles. After the last matmul
            # only two parallel 128-wide evictions and one 64KB DMA on
            # the idle sync ring remain.
            db = DB - 1
            cks = chunk_of_db[db]
            row0 = db * P
            if len(cks) == 3 and cks[1][1] == NFREE:
                (o0, cw0), (o1, cw1), (o2, cw2) = cks
                h = cw1 // 2
                hh = h // 2
                sub = [(o0, cw0), (o2, cw2), (o1, h),
                       (o1 + h, hh), (o1 + h + hh, cw1 - h - hh)]
                ps = {j: pspool.tile([P, cw], dt.float32, name=f"ps15_{j}", tag="ps")
                      for j, (off, cw) in enumerate(sub)}
                ysb = new_ysb(db)

                def mm15(j, ki):
                    off, cw = sub[j]
                    nc.tensor.matmul(
                        ps[j][:, :cw], lhsT(db, ki), x_sl(ki, off, cw),
                        start=(ki == 0), stop=(ki == KT - 1),
                    )

                for ki in range(KT):
                    mm15(0, ki)
                    mm15(1, ki)
                ev_act(ysb[:, o0:o0 + cw0], ps[0][:, :cw0], db)
                ev_dve(ysb[:, o2:o2 + cw2], ps[1][:, :cw2], db)
                nc.sync.dma_start(y_d[row0:row0 + P, o0:o0 + cw0],
                                  ysb[:, o0:o0 + cw0])
                nc.scalar.dma_start(y_d[row0:row0 + P, o2:o2 + cw2],
                                    ysb[:, o2:o2 + cw2])
                for ki in range(KT):
                    mm15(2, ki)
                # c1a on the DVE so the ACT is guaranteed free for the
                # final sub3 eviction the moment the last matmul ends
                ev_dve(ysb[:, o1:o1 + h], ps[2][:, :h], db)
                # c1a's 256KB transfer rides the scalar ring so the
                # final 64KB piece on sync doesn't queue behind it.
                nc.scalar.dma_start(y_d[row0:row0 + P, o1:o1 + h],
                                    ysb[:, o1:o1 + h])
                for ki in range(KT):
                    mm15(3, ki)
                    mm15(4, ki)
                o3, cw3 = sub[3]
                o4, cw4 = sub[4]
                ev_act(ysb[:, o3:o3 + cw3], ps[3][:, :cw3], db)
                ev_dve(ysb[:, o4:o4 + cw4], ps[4][:, :cw4], db)
                nc.sync.dma_start(y_d[row0:row0 + P, o3:o3 + cw3 + cw4],
                                  ysb[:, o3:o3 + cw3 + cw4])
            else:
                # generic fallback (different C): plain chunk-major
                alloc_psums(db)
                for ki in range(KT):
                    emit_mm(db, ki, 0)
                ysb = new_ysb(db)
                evict_chunk(db, 0, ysb)
                for ki in range(KT):
                    for j in range(1, len(cks)):
                        emit_mm(db, ki, j)
                for j in range(1, len(cks)):
                    evict_chunk(db, j, ysb)
                for j, (off, cw) in enumerate(cks):
                    eng = nc.sync if j % 2 == 0 else nc.scalar
                    eng.dma_start(
                        y_d[row0:row0 + P, off:off + cw], ysb[:, off:off + cw]
                    )

    nc.compile()
    return nc


def _run_spmd(in_maps, C, trace=False, trace_cores=None):
    from concourse.bass_utils import run_bass_kernel_spmd

    nc = _nc_cache.get(C)
    if nc is None:
        nc = _build_bass(C)
        _nc_cache[C] = nc
    return run_bass_kernel_spmd(
        nc,
        in_maps,
        core_ids=list(range(N_CORES)),
        trace=trace,
        trace_cores=trace_cores,
    )


def _prepare(x, weight, bias, ids_sorted):
    """Host-side routing + int8 weight quantization.

    Returns (in_maps, C, counts, starts)."""
    x = np.asarray(x)
    weight = np.asarray(weight)
    bias = np.asarray(bias)
    ids = np.asarray(ids_sorted)

    counts = np.bincount(ids, minlength=N_EXP).astype(np.int64)
    starts = np.zeros(N_EXP, dtype=np.int64)
    starts[1:] = np.cumsum(counts)[:-1]
    C = max(int(counts.max()), 2)
    C += C % 2

    KT = D_IN // P
    DB = D_OUT // P
    KW = KT * P

    xb = x.astype(BF16)

    in_maps = []
    for e in range(N_EXP):
        n_e = int(counts[e])
        xeT = np.zeros((D_IN, C), dtype=BF16)
        if n_e:
            xeT[:, :n_e] = xb[starts[e]:starts[e] + n_e].T
        # partition-flat x: row p, col ki*C + c = x[token c, ki*128+p]
        xf = np.ascontiguousarray(
            xeT.reshape(KT, P, C).transpose(1, 0, 2)
        ).reshape(P, KT * C)

        we = weight[e]  # [d_out, d_in] f32
        # dbs 0/1: bf16, unquantized
        we01T = we[:2 * P].T.astype(BF16)  # [d_in, 256]
        w01 = np.ascontiguousarray(
            we01T.reshape(KT, P, 2, P).transpose(1, 2, 0, 3)
        ).reshape(P, 2 * KW)
        # dbs 2..15: int8 with per-output-row scale
        wrest = we[2 * P:]  # [d_out-256, d_in]
        s = np.abs(wrest).max(axis=1).astype(np.float32) / np.float32(127.0)
        s = np.maximum(s, np.float32(1e-30))
        w8 = np.rint(wrest / s[:, None]).clip(-127, 127).astype(np.int8)
        w8T = w8.T  # [d_in, d_out-256]
        w8f = np.ascontiguousarray(
            w8T.reshape(KT, P, DB - 2, P).transpose(1, 2, 0, 3)
        ).reshape(P, (DB - 2) * KW)

        # eviction scale column: dbs 0/1 -> 1.0; dbs 2+ -> s[d]
        sc = np.empty(D_OUT, dtype=np.float32)
        sc[:2 * P] = 1.0
        sc[2 * P:] = s
        scp = np.ascontiguousarray(sc.reshape(DB, P).T)
        bp = np.ascontiguousarray(
            bias[e].astype(BF16).astype(np.float32).reshape(DB, P).T
        )
        in_maps.append(
            {"xf": xf, "w01": w01, "w8f": w8f, "biasp": bp, "scp": scp}
        )
    return in_maps, C, counts, starts


def _assemble(results, counts, starts):
    out = np.empty((N_TOK, D_OUT), dtype=BF16)
    for e in range(N_EXP):
        n_e = int(counts[e])
        if n_e:
            out[starts[e]:starts[e] + n_e] = results[e]["yT"][:, :n_e].T
    return out


def kernel(x, weight, bias, ids_sorted):
    in_maps, C, counts, starts = _prepare(x, weight, bias, ids_sorted)
    res = _run_spmd(in_maps, C)
    return _assemble(res.results, counts, starts)


# revision 24
# speedup vs baseline: 1.0238x; 1.0238x over previous
"""MoE grouped linear (DMoELinear) on 8 Trainium2 NeuronCores.

Expert-parallel sharding: tokens are sorted by expert id, so expert e's
tokens form one contiguous slice. Core e receives expert e's tokens
(padded to a uniform capacity C = max group size, so all cores run one
SPMD NEFF), expert e's weight and bias, and computes
    yT_e = (x_e @ W_e.T).T.bf16 + b_e.bf16
with the weight block as the stationary matmul operand and tokens as
the moving free dim.

v9 schedule (trace-driven). Measured constraints: each dma_start costs
~0.6us of issuing-engine time; each engine rotates 4 DMA queues (issue
N+4 blocks on N's completion); per-ring throughput under 8-core HBM
contention is ~175-230GB/s. The 3.2MB trickle stream (x + w0/w1) is
the binding constraint for the first ~17us, so it rides few, large
transfers interleaved by first-use deadline; w2 follows split in
halves (db2 runs k-major so it only needs w2k0 at the trickle's
drain), then w3 and the 2-db packs with multi-us slack. (int8 weight
shipping + on-chip upconvert was tried and abandoned: the only fast
int8->bf16 path is ACTIVATE-Copy on the ACT engine (~115G elem/s; the
DVE/Pool tensor_scalar path measures ~9G under load) and it cannot
pace the trickle, while the late tensors it can pace were never
deadline-bound.)

PE schedule: 32 warm matmuls flip the HAM clock gate while the first
tiles land (the real stream gates the start at ~11.5 anyway); db0/db1
interleave k-major (trickle, DMA-paced); db2 runs k-major; dbs 3..14
run chunk-major (c0's whole k-loop first, evicted while c1/c2 —
interleaved so the narrow chunk's LDWEIGHTS hide under 512-wide
drains — still compute) so PSUM banks retire mid-db; db15 splits its
middle 512 region into 256+128+128 pieces with per-piece PSUM tiles
(shared psum tiles serialize readers in the dep tracker) so after the
last matmul only two parallel 128-wide evictions and one 64KB DMA
remain. Bias adds fuse into the PSUM evictions (ACT/DVE alternating).
"""

import numpy as np
import ml_dtypes

N_TOK, D_IN, D_OUT, N_EXP = 8192, 1024, 2048, 8
N_CORES = 8
P = 128
NFREE = 512  # max matmul moving free dim (one PSUM bank of f32)

BF16 = ml_dtypes.bfloat16

_nc_cache: dict[int, object] = {}


def _chunks(C):
    out = []
    off = 0
    while off < C:
        cw = min(NFREE, C - off)
        out.append((off, cw))
        off += cw
    return out


def _build_bass(C: int):
    """Emit the per-core Bass/Tile kernel for token capacity C."""
    import concourse.bass as bass  # noqa: F401  (registers engines)
    import concourse.mybir as mybir
    import concourse.tile as tile
    from concourse import bacc

    dt = mybir.dt
    KT = D_IN // P      # 8 contraction tiles
    DB = D_OUT // P     # 16 output-row blocks
    KW = KT * P         # columns per db block in the flat weight (1024)
    chunks = _chunks(C)
    chunk_of_db = {db: chunks for db in range(DB)}

    nc = bacc.Bacc("TRN2", target_bir_lowering=False)

    # x partition-flat: row p, col ki*C + c  =  x[token c, ki*128+p]
    xf_d = nc.dram_tensor("xf", [P, KT * C], dt.bfloat16, kind="ExternalInput")
    # flat weights: row p, col db*1024 + kt*128 + d  (lhsT slices are
    # contiguous 128-col blocks; multi-db packs are contiguous too).
    wf_d = nc.dram_tensor("wf", [P, DB * KW], dt.bfloat16, kind="ExternalInput")
    bias_d = nc.dram_tensor("biasp", [P, DB], dt.float32, kind="ExternalInput")
    y_d = nc.dram_tensor("yT", [D_OUT, C], dt.bfloat16, kind="ExternalOutput")

    with tile.TileContext(nc) as tc:
        with (
            tc.tile_pool(name="persist", bufs=1) as ppool,
            tc.tile_pool(name="yout", bufs=4) as ypool,
            tc.tile_pool(name="psum", bufs=8, space="PSUM") as pspool,
        ):
            x_tiles = [
                ppool.tile([P, C], dt.bfloat16, name=f"x{ki}", tag=f"x{ki}")
                for ki in range(KT)
            ]

            def x_sl(ki, off, cw):
                return x_tiles[ki][:, off:off + cw]

            w_s = [
                ppool.tile([P, KW], dt.bfloat16, name=f"w{db}", tag=f"w{db}")
                for db in range(4)
            ]
            packs = [
                ppool.tile([P, 2 * KW], dt.bfloat16, name=f"wp{g}", tag=f"wp{g}")
                for g in range(2, 8)
            ]
            bt = ppool.tile([P, DB], dt.float32, name="bias", tag="bias")

            def lhsT(db, ki):
                if db < 4:
                    return w_s[db][:, ki * P:(ki + 1) * P]
                g = db // 2
                off = (db - 2 * g) * KW + ki * P
                return packs[g - 2][:, off:off + P]

            # ── DMA schedule: two HWDGE rings ────────────────────────
            A, B = nc.sync, nc.scalar

            def xdma(ki, eng):
                eng.dma_start(x_tiles[ki][:], xf_d[:, ki * C:(ki + 1) * C])

            xdma(1, A)
            xdma(0, B)
            A.dma_start(w_s[0][:], wf_d[:, 0:KW])
            B.dma_start(w_s[1][:], wf_d[:, KW:2 * KW])
            xdma(3, A)
            xdma(2, B)
            xdma(5, A)
            xdma(4, B)
            xdma(7, A)
            xdma(6, B)
            B.dma_start(bt[:], bias_d[:])
            A.dma_start(w_s[2][:, 0:4 * P], wf_d[:, 2 * KW:2 * KW + 4 * P])
            A.dma_start(w_s[2][:, 4 * P:8 * P], wf_d[:, 2 * KW + 4 * P:3 * KW])
            B.dma_start(w_s[3][:], wf_d[:, 3 * KW:4 * KW])
            for g in range(2, 8):
                eng = B if g % 2 == 0 else A
                eng.dma_start(packs[g - 2][:], wf_d[:, 2 * g * KW:(2 * g + 2) * KW])

            # ── PE warmup: flip the HAM clock gate (~3.5us of activity)
            # while the first DMAs land. The first real matmul is gated
            # by the w0 slab landing ~11.5 anyway.
            warm = ppool.tile([P, P], dt.bfloat16, name="warm", tag="warm")
            nc.vector.memset(warm[:], 0.0)
            wps = pspool.tile([P, P], dt.float32, name="wps", tag="ps")

            def warm_mm(n):
                for _ in range(n):
                    nc.tensor.matmul(wps[:], warm[:], warm[:], start=True, stop=True)

            warm_mm(32)

            all_psums = {}

            def alloc_chunk(db, j):
                _, cw = chunk_of_db[db][j]
                return pspool.tile([P, cw], dt.float32, name=f"ps{db}_{j}", tag="ps")

            def alloc_psums(db, chunks_j=None):
                js = chunks_j or range(len(chunk_of_db[db]))
                cur = all_psums.setdefault(db, {})
                for j in js:
                    cur[j] = alloc_chunk(db, j)

            def emit_mm(db, ki, j):
                off, cw = chunk_of_db[db][j]
                nc.tensor.matmul(
                    all_psums[db][j][:, :cw],
                    lhsT(db, ki),
                    x_sl(ki, off, cw),
                    start=(ki == 0),
                    stop=(ki == KT - 1),
                )

            def emit_mms(db, ki, chunks_j=None):
                for j in chunks_j or range(len(chunk_of_db[db])):
                    emit_mm(db, ki, j)

            ep = 0
            ysbs = {}

            def new_ysb(db):
                ysbs[db] = ypool.tile([P, C], dt.bfloat16, name="ysb", tag="ysb")
                return ysbs[db]

            def evict_chunk(db, j, ysb):
                nonlocal ep
                off, cw = chunk_of_db[db][j]
                bias_col = bt[:, db:db + 1]
                if ep % 2 == 0:
                    nc.scalar.add(ysb[:, off:off + cw], all_psums[db][j][:, :cw], bias_col)
                else:
                    nc.vector.tensor_scalar_add(
                        ysb[:, off:off + cw], all_psums[db][j][:, :cw], bias_col
                    )
                ep += 1

            def evict(db):
                ysb = new_ysb(db)
                for j in range(len(chunk_of_db[db])):
                    evict_chunk(db, j, ysb)
                return ysb

            def ydma(db, ysb):
                eng = nc.sync if db % 2 == 0 else nc.scalar
                eng.dma_start(y_d[db * P:(db + 1) * P, :], ysb[:])

            # ── Trickle phase ────────────────────────────────────────
            # db0/db1 interleaved by k-step; db1 one step behind so
            # db0's k7 chunks finish (and their PSUM banks evict) while
            # db1's tail runs.
            STAG = 1
            alloc_psums(0)
            alloc_psums(1)
            for step in range(KT + STAG):
                if step < KT:
                    emit_mms(0, step)
                if step >= STAG:
                    emit_mms(1, step - STAG)
            ydma(0, evict(0))
            ydma(1, evict(1))

            # db2 stays k-major: it starts right at the trickle's drain
            # and k-major only needs w2k0 by then (chunk-major would
            # need all of w2, which is still streaming).
            alloc_psums(2)
            for ki in range(KT):
                emit_mms(2, ki)
            ydma(2, evict(2))

            # ── dbs 3..14: chunk-major ───────────────────────────────
            # c0 runs its whole k-loop first and evicts while c1/c2
            # (interleaved so the narrow chunk's LDWEIGHTS hide under
            # the 512-wide drains) are still computing. PSUM banks
            # retire mid-db instead of piling up at db boundaries.
            for db in range(3, DB - 1):
                alloc_psums(db)
                ncks = len(chunk_of_db[db])
                for ki in range(KT):
                    emit_mm(db, ki, 0)
                ysb = new_ysb(db)
                evict_chunk(db, 0, ysb)
                for ki in range(KT):
                    for j in range(1, ncks):
                        emit_mm(db, ki, j)
                for j in range(1, ncks):
                    evict_chunk(db, j, ysb)
                if db == DB - 2:
                    # per-chunk DMAs on both rings so the tail pipelines
                    for j, (off, cw) in enumerate(chunk_of_db[db]):
                        eng = nc.sync if (db + j) % 2 == 0 else nc.scalar
                        eng.dma_start(
                            y_d[db * P:(db + 1) * P, off:off + cw],
                            ysb[:, off:off + cw],
                        )
                else:
                    ydma(db, ysb)

            # ── db15: ordered so the drain is minimal ────────────────
            # (c0, tail-chunk) interleaved first — both evicted and
            # DMA'd while the middle 512 region computes as 256+128+128
            # pieces with per-piece PSUM tiles. After the last matmul
            # only two parallel 128-wide evictions and one 64KB DMA on
            # the sync ring remain (c1a's 256KB rides the scalar ring
            # so the final piece doesn't queue behind it).
            db = DB - 1
            cks = chunk_of_db[db]
            row0 = db * P
            bias_col = bt[:, db:db + 1]
            if len(cks) == 3 and cks[1][1] == NFREE:
                (o0, cw0), (o1, cw1), (o2, cw2) = cks
                h = cw1 // 2
                hh = h // 2
                sub = [(o0, cw0), (o2, cw2), (o1, h),
                       (o1 + h, hh), (o1 + h + hh, cw1 - h - hh)]
                ps = {j: pspool.tile([P, cw], dt.float32, name=f"ps15_{j}", tag="ps")
                      for j, (off, cw) in enumerate(sub)}
                ysb = new_ysb(db)

                def mm15(j, ki):
                    off, cw = sub[j]
                    nc.tensor.matmul(
                        ps[j][:, :cw], lhsT(db, ki), x_sl(ki, off, cw),
                        start=(ki == 0), stop=(ki == KT - 1),
                    )

                for ki in range(KT):
                    mm15(0, ki)
                    mm15(1, ki)
                nc.scalar.add(ysb[:, o0:o0 + cw0], ps[0][:, :cw0], bias_col)
                nc.vector.tensor_scalar_add(
                    ysb[:, o2:o2 + cw2], ps[1][:, :cw2], bias_col
                )
                nc.sync.dma_start(y_d[row0:row0 + P, o0:o0 + cw0],
                                  ysb[:, o0:o0 + cw0])
                nc.scalar.dma_start(y_d[row0:row0 + P, o2:o2 + cw2],
                                    ysb[:, o2:o2 + cw2])
                for ki in range(KT):
                    mm15(2, ki)
                nc.vector.tensor_scalar_add(
                    ysb[:, o1:o1 + h], ps[2][:, :h], bias_col
                )
                nc.scalar.dma_start(y_d[row0:row0 + P, o1:o1 + h],
                                    ysb[:, o1:o1 + h])
                for ki in range(KT):
                    mm15(3, ki)
                    mm15(4, ki)
                o3, cw3 = sub[3]
                o4, cw4 = sub[4]
                nc.scalar.add(ysb[:, o3:o3 + cw3], ps[3][:, :cw3], bias_col)
                nc.vector.tensor_scalar_add(
                    ysb[:, o4:o4 + cw4], ps[4][:, :cw4], bias_col
                )
                nc.sync.dma_start(y_d[row0:row0 + P, o3:o3 + cw3 + cw4],
                                  ysb[:, o3:o3 + cw3 + cw4])
            else:
                # generic fallback (different C): plain chunk-major
                alloc_psums(db)
                for ki in range(KT):
                    emit_mm(db, ki, 0)
                ysb = new_ysb(db)
                evict_chunk(db, 0, ysb)
                for ki in range(KT):
                    for j in range(1, len(cks)):
                        emit_mm(db, ki, j)
                for j in range(1, len(cks)):
                    evict_chunk(db, j, ysb)
                for j, (off, cw) in enumerate(cks):
                    eng = nc.sync if j % 2 == 0 else nc.scalar
                    eng.dma_start(
                        y_d[row0:row0 + P, off:off + cw], ysb[:, off:off + cw]
                    )

    nc.compile()
    return nc


def _run_spmd(in_maps, C, trace=False, trace_cores=None):
    from concourse.bass_utils import run_bass_kernel_spmd

    nc = _nc_cache.get(C)
    if nc is None:
        nc = _build_bass(C)
        _nc_cache[C] = nc
    return run_bass_kernel_spmd(
        nc,
        in_maps,
        core_ids=list(range(N_CORES)),
        trace=trace,
        trace_cores=trace_cores,
    )


def _prepare(x, weight, bias, ids_sorted):
    """Host-side routing: returns (in_maps, C, counts, starts)."""
    x = np.asarray(x)
    weight = np.asarray(weight)
    bias = np.asarray(bias)
    ids = np.asarray(ids_sorted)

    counts = np.bincount(ids, minlength=N_EXP).astype(np.int64)
    starts = np.zeros(N_EXP, dtype=np.int64)
    starts[1:] = np.cumsum(counts)[:-1]
    C = max(int(counts.max()), 2)
    C += C % 2

    KT = D_IN // P
    DB = D_OUT // P
    xb = x.astype(BF16)
    in_maps = []
    for e in range(N_EXP):
        n_e = int(counts[e])
        xeT = np.zeros((D_IN, C), dtype=BF16)
        if n_e:
            xeT[:, :n_e] = xb[starts[e]:starts[e] + n_e].T
        # partition-flat x: row p, col ki*C + c = x[token c, ki*128+p]
        xf = np.ascontiguousarray(
            xeT.reshape(KT, P, C).transpose(1, 0, 2)
        ).reshape(P, KT * C)
        # flat weight: row p, col db*1024 + kt*128 + d  = W_e[db*128+d, kt*128+p]
        weT = weight[e].T.astype(BF16)  # [d_in, d_out]
        wf = np.ascontiguousarray(
            weT.reshape(KT, P, DB, P).transpose(1, 2, 0, 3)
        ).reshape(P, DB * KT * P)
        bp = np.ascontiguousarray(
            bias[e].astype(BF16).astype(np.float32).reshape(DB, P).T
        )
        in_maps.append({"xf": xf, "wf": wf, "biasp": bp})
    return in_maps, C, counts, starts


def _assemble(results, counts, starts):
    out = np.empty((N_TOK, D_OUT), dtype=BF16)
    for e in range(N_EXP):
        n_e = int(counts[e])
        if n_e:
            out[starts[e]:starts[e] + n_e] = results[e]["yT"][:, :n_e].T
    return out


def kernel(x, weight, bias, ids_sorted):
    in_maps, C, counts, starts = _prepare(x, weight, bias, ids_sorted)
    res = _run_spmd(in_maps, C)
    return _assemble(res.results, counts, starts)
